# revision 1
# baseline (speedup 1.0000x reference)
"""FBPinn forward kernel for Trainium2 (8 NeuronCores, Bass/Tile).

The module computes y(x) = tanh(x) * sum_w [win_w(x)>1e-3] * win_w(x) * MLP_w(x)
for 1M scalar points x in [0,100) -- a fixed 1D function of x. Per core:
  1. evaluate the function at the 4097 knots of a uniform grid over the core's
     12.5-wide domain slice (32768 cells total) using the 30 tiny MLPs
     (block-diagonal-packed PE matmuls, tanh/sigmoid on ACT), masking windows
     exactly at each knot;
  2. assemble per-cell linear records (a0,b0,a1,b1,xsplit) -- two-sided at the
     54 win==1e-3 mask discontinuities so the jumps are reproduced exactly --
     entirely in SBUF, aligned so partition p owns cells [32p, 32p+32);
  3. points are packed (host side) into a (partition, cell)-aligned slot grid:
     cell c -> partition c//32, block c%32, S slots per cell. Interpolation is
     then pure elementwise DVE work with stride-0 broadcast reads of the
     records -- no gather at all.
Host shards points by domain across the 8 cores, packs slots, and un-permutes
the outputs. Piecewise-linear error on this grid is ~2e-6 absmax.
"""

import numpy as np

# ---------------- problem constants (hardcoded from the module spec) ----------
NW = 30
DOM0, DOM1 = 0.0, 100.0
OVERLAP = 0.25
NEURONS = 32
THRESH = 0.001
N = 1_000_000

NCORES = 8
P = 128                      # SBUF partitions
CPP = 24                     # cells per partition
C_LOC = P * CPP              # cells per core (4096)
DW = 12.5                    # per-core domain width
H = DW / C_LOC               # cell width (exact in fp32)
INVH = C_LOC / DW
NG = 3                       # window groups of 4 per core
NSLOT = 4 * NG               # window slots per core
KCHUNK = 512                 # knot columns per pipeline chunk
NKFULL = C_LOC // KCHUNK     # full chunks (6)
KCH_LAST = 128               # short final chunk (last knot + specials + pad)
NKCH = NKFULL + 1
KT = KCHUNK * NKFULL + KCH_LAST
NKNOT = C_LOC + 1            # real grid knots
NB = 16                      # straddle-boundary slots per core
SPEC0 = C_LOC + 8            # first special knot column
S_DEFAULT = 76               # point slots per cell
BIG = np.float32(1e30)


# ---------------- geometry (host, input-independent) --------------------------
def _partition_geom():
    width = (DOM1 - DOM0) / NW
    sub = np.zeros((NW, 2), np.float32)
    for i in range(NW):
        sub[i, 0] = DOM0 if i == 0 else DOM0 + (i - OVERLAP / 2) * width
        sub[i, 1] = DOM1 if i == NW - 1 else DOM0 + (i + 1 + OVERLAP / 2) * width
    means = (sub[:, 0] + sub[:, 1]) / 2
    std = (sub[:, 1] - sub[:, 0]) / 2
    mid = np.zeros(NW + 1, np.float32)
    mid[0] = sub[0, 0]
    mid[-1] = sub[-1, 1]
    for i in range(1, NW):
        mid[i] = (sub[i - 1, 1] + sub[i, 0]) / 2
    return means.astype(np.float32), std.astype(np.float32), mid.astype(np.float32)


def _win64(l, r, x):
    return 1.0 / (1 + np.exp(-(x - l))) / (1 + np.exp(x - r))


def _bisect64(l, r, lo, hi, rising):
    for _ in range(200):
        m = 0.5 * (lo + hi)
        if (_win64(l, r, m) < THRESH) == rising:
            lo = m
        else:
            hi = m
    return 0.5 * (lo + hi)


def _refine_flip_fp32(l32, r32, b64, rising):
    """Exact fp32 x where the reference's jax-fp32 predicate win(x)>1e-3 flips.
    Returns the smallest fp32 x at which the predicate equals its right-side
    state. Falls back to the float64 bisection value if jax is unavailable."""
    try:
        import jax
        import jax.numpy as jnp

        cpu = jax.devices("cpu")[0]
        lo = np.float32(b64 - 5e-5)
        hi = np.float32(b64 + 5e-5)
        xs = np.arange(lo.view(np.int32), hi.view(np.int32) + 1,
                       dtype=np.int32).view(np.float32)
        with jax.default_device(cpu):
            win = np.asarray(
                jax.nn.sigmoid(jnp.asarray(xs) - np.float32(l32))
                * jax.nn.sigmoid(-(jnp.asarray(xs) - np.float32(r32)))
            )
        pred = win > np.float32(THRESH)
        state = pred if rising else ~pred
        if not state.any() or state.all():
            return np.float32(b64)
        k = int(np.argmax(state))
        if not state[k:].all():
            return np.float32(b64)
        return xs[k]
    except Exception:
        return np.float32(b64)


_GEOM = None


def _geometry():
    global _GEOM
    if _GEOM is not None:
        return _GEOM
    means, std, mid = _partition_geom()
    ml = mid[:-1].astype(np.float64)
    mr = mid[1:].astype(np.float64)
    Lb = np.zeros(NW, np.float32)   # window-on lower bound (exact fp32 flip)
    Rb = np.zeros(NW, np.float32)   # window-off upper bound
    for w in range(NW):
        c = 0.5 * (ml[w] + mr[w])
        l64 = _bisect64(ml[w], mr[w], ml[w] - 30, c, rising=True)
        r64 = _bisect64(ml[w], mr[w], c, mr[w] + 30, rising=False)
        Lb[w] = _refine_flip_fp32(mid[w], mid[w + 1], l64, rising=True)
        Rb[w] = _refine_flip_fp32(mid[w], mid[w + 1], r64, rising=False)
    bnds = []
    for w in range(NW):
        if DOM0 < Lb[w] < DOM1:
            bnds.append(float(Lb[w]))
        if DOM0 < Rb[w] < DOM1:
            bnds.append(float(Rb[w]))
    bnds = np.sort(np.array(bnds, np.float64))
    _GEOM = (means, std, mid, Lb, Rb, bnds)
    return _GEOM




def _group_chunk_need():
    """need[ch][g]: does group g contribute anywhere in knot-chunk ch for ANY
    core? Computed from input-independent window geometry."""
    means, std, mid, Lb, Rb, bnds = _geometry()
    need = [[False] * NG for _ in range(NKCH)]
    for core in range(NCORES):
        base = DOM0 + core * DW
        act = [w for w in range(NW) if (Rb[w] > base) and (Lb[w] < base + DW)]
        for slot, w in enumerate(act):
            g = slot // 4
            lo, hi = float(Lb[w]) - base, float(Rb[w]) - base
            for ch in range(NKCH):
                c_lo = ch * KCHUNK * H
                c_hi = (ch + 1) * KCHUNK * H
                if ch == NKCH - 1:
                    c_hi = 1e30    # specials chunk: straddle x anywhere
                    c_lo = 0.0
                if hi > c_lo - 4 * H and lo < c_hi + 4 * H:
                    need[ch][g] = True
    return need

# ---------------- bass program (built once per S, SPMD across 8 cores) --------
_PROGS = {}


def _build_program(S):
    if S in _PROGS:
        return _PROGS[S]
    from concourse import bacc, bass, mybir, tile
    from concourse.bass import IndirectOffsetOnAxis

    f32 = mybir.dt.float32
    f32r = mybir.dt.float32r
    i32 = mybir.dt.int32
    u8 = mybir.dt.uint8
    Act = mybir.ActivationFunctionType
    Op = mybir.AluOpType

    M = CPP * S
    PBLK = 8                      # cell-blocks per point chunk
    PCH = PBLK * S                # point columns per chunk
    NPCH = CPP // PBLK

    nc = bacc.Bacc(None, target_bir_lowering=False)

    x_in = nc.declare_dram_parameter("x_pts", [P, M], f32, isOutput=False)
    base_in = nc.declare_dram_parameter("base_col", [P, 1], f32, isOutput=False)
    c0p_in = nc.declare_dram_parameter("c0p_col", [P, 1], f32, isOutput=False)
    sc1_in = nc.declare_dram_parameter("sc1", [P, NG], f32, isOutput=False)
    bi1_in = nc.declare_dram_parameter("bi1", [P, NG], f32, isOutput=False)
    w2_in = nc.declare_dram_parameter("w2blk", [P, P * NG], f32, isOutput=False)
    w3_in = nc.declare_dram_parameter("w3f", [P, NSLOT * NG], f32, isOutput=False)
    b2_in = nc.declare_dram_parameter("b2c", [P, NG], f32, isOutput=False)
    b3_in = nc.declare_dram_parameter("b3c", [NSLOT, 1], f32, isOutput=False)
    negl_in = nc.declare_dram_parameter("negl", [NSLOT, 1], f32, isOutput=False)
    rr_in = nc.declare_dram_parameter("rr", [NSLOT, 1], f32, isOutput=False)
    lb_in = nc.declare_dram_parameter("lbnd", [NSLOT, 1], f32, isOutput=False)
    rb_in = nc.declare_dram_parameter("rbnd", [NSLOT, 1], f32, isOutput=False)
    knots_in = nc.declare_dram_parameter("knotrep", [P, KT], f32, isOutput=False)
    k12_in = nc.declare_dram_parameter("knot12", [NSLOT, KT], f32, isOutput=False)
    xs_in = nc.declare_dram_parameter("xsplit_col", [P, CPP], f32, isOutput=False)
    itb_in = nc.declare_dram_parameter("invtb_col", [P, CPP], f32, isOutput=False)
    i1tb_in = nc.declare_dram_parameter("inv1mtb_col", [P, CPP], f32, isOutput=False)
    jl_in = nc.declare_dram_parameter("jlist", [NB, 1], i32, isOutput=False)
    wm_in = nc.declare_dram_parameter("wmask", [NSLOT, KT], f32, isOutput=False)
    on_in = nc.declare_dram_parameter("ones12", [NSLOT, 1], f32, isOutput=False)
    y_out = nc.declare_dram_parameter("y_out", [P, M], f32, isOutput=True)

    with tile.TileContext(nc) as tc:
        with (
            tc.tile_pool(name="const", bufs=1) as cpool,
            tc.tile_pool(name="work", bufs=2) as wpool,
            tc.tile_pool(name="pts", bufs=2) as ppool,
            tc.tile_pool(name="psum", bufs=2, space="PSUM") as psum,
            tc.tile_pool(name="dram", bufs=1, space="DRAM") as dpool,
        ):
            v_dram = dpool.tile([KT, 1], f32)            # knot values
            vm_dram = dpool.tile([C_LOC + NB, 1], f32)   # left-limit knot col
            vp_dram = dpool.tile([C_LOC + NB, 1], f32)   # right-limit knot col

            _eng = [nc.sync, nc.scalar, nc.gpsimd]
            _eng_i = [0]

            def load(handle, shape, tag, dtype=f32, eng=None):
                t = cpool.tile(shape, dtype, tag=tag)
                e = eng if eng is not None else _eng[_eng_i[0] % len(_eng)]
                _eng_i[0] += 1
                e.dma_start(out=t[:], in_=handle[:])
                return t

            xp = load(x_in, [P, M], "c_x", eng=nc.gpsimd)
            base_c = load(base_in, [P, 1], "c_base")
            c0p = load(c0p_in, [P, 1], "c_c0p")
            sc1 = load(sc1_in, [P, NG], "c_sc1")
            bi1 = load(bi1_in, [P, NG], "c_bi1")
            w2 = load(w2_in, [P, P * NG], "c_w2")
            w3 = load(w3_in, [P, NSLOT * NG], "c_w3")
            b2 = load(b2_in, [P, NG], "c_b2")
            b3 = load(b3_in, [NSLOT, 1], "c_b3")
            negl = load(negl_in, [NSLOT, 1], "c_negl")
            rr = load(rr_in, [NSLOT, 1], "c_rr")
            k12 = load(k12_in, [NSLOT, KT], "c_k12", eng=nc.scalar)
            knots = load(knots_in, [P, KT], "c_knots", eng=nc.sync)
            xs_c = load(xs_in, [P, CPP], "c_xs")
            itb = load(itb_in, [P, CPP], "c_itb")
            i1tb = load(i1tb_in, [P, CPP], "c_i1tb")
            jl = load(jl_in, [NB, 1], "c_jl", i32)
            wmask = load(wm_in, [NSLOT, KT], "c_wm", eng=nc.scalar)

            ones12 = load(on_in, [NSLOT, 1], "c_ones")
            jrow_i = cpool.tile([P, M], i32, tag="c_jri")
            nc.gpsimd.iota(
                jrow_i[:].rearrange("p (c s) -> p c s", c=CPP),
                pattern=[[1, CPP], [0, S]], channel_multiplier=0,
            )
            jrow = cpool.tile([P, M], f32, tag="c_jrf")
            nc.vector.tensor_copy(out=jrow[:], in_=jrow_i[:])

            # ---- phase B: knot values ----
            need = _group_chunk_need()
            # hoist all sigmoids + the win*mask product out of the chunk loop
            # (avoids per-chunk ACT table reloads between Tanh and Sigmoid)
            s1a = cpool.tile([NSLOT, KT], f32, tag="s1a")
            nc.scalar.activation(out=s1a[:], in_=k12[:],
                                 func=Act.Sigmoid, bias=negl[:], scale=1.0)
            s2a = cpool.tile([NSLOT, KT], f32, tag="s2a")
            nc.scalar.activation(out=s2a[:], in_=k12[:],
                                 func=Act.Sigmoid, bias=rr[:], scale=-1.0)
            wina = cpool.tile([NSLOT, KT], f32, tag="wina")
            nc.vector.tensor_mul(out=wina[:], in0=s1a[:], in1=s2a[:])
            nc.vector.tensor_mul(out=wina[:], in0=wina[:], in1=wmask[:])
            tha = cpool.tile([1, KT], f32, tag="tha")
            nc.scalar.activation(out=tha[:], in_=k12[0:1, :], func=Act.Tanh)
            for ch in range(NKCH):
                kw = KCHUNK if ch < NKFULL else KCH_LAST
                sl = slice(ch * KCHUNK, ch * KCHUNK + kw)
                xk = knots[:, sl]
                gs = [g for g in range(NG) if need[ch][g]]
                h2s = {}
                for g in gs:
                    h1 = wpool.tile([P, KCHUNK], f32, tag="h1")
                    nc.scalar.activation(
                        out=h1[:, :kw], in_=xk, func=Act.Tanh,
                        bias=bi1[:, g : g + 1], scale=sc1[:, g : g + 1],
                    )
                    h2p = psum.tile([P, KCHUNK], f32, tag="h2p")
                    nc.tensor.matmul(
                        out=h2p[:, :kw], lhsT=w2[:, g * P : (g + 1) * P],
                        rhs=h1[:, :kw], start=True, stop=True,
                    )
                    h2 = wpool.tile([P, KCHUNK], f32, tag=f"h2_{g}")
                    nc.scalar.activation(
                        out=h2[:, :kw], in_=h2p[:, :kw], func=Act.Tanh,
                        bias=b2[:, g : g + 1], scale=1.0,
                    )
                    h2s[g] = h2
                pre = psum.tile([NSLOT, KCHUNK], f32, tag="pre")
                for i, g in enumerate(gs):
                    nc.tensor.matmul(
                        out=pre[:, :kw],
                        lhsT=w3[:, g * NSLOT : (g + 1) * NSLOT],
                        rhs=h2s[g][:, :kw], start=(i == 0), stop=(i == len(gs) - 1),
                    )
                term = wpool.tile([NSLOT, KCHUNK], f32, tag="term")
                nc.vector.tensor_scalar(out=term[:, :kw], in0=pre[:, :kw],
                                        scalar1=b3[:], scalar2=None, op0=Op.add)
                nc.vector.tensor_mul(out=term[:, :kw], in0=term[:, :kw],
                                     in1=wina[:, sl])
                vp_ps = psum.tile([1, KCHUNK], f32, tag="vp")
                nc.tensor.matmul(out=vp_ps[:, :kw], lhsT=ones12[:],
                                 rhs=term[:, :kw], start=True, stop=True)
                vrow = wpool.tile([1, KCHUNK], f32, tag="vrow")
                nc.vector.tensor_mul(out=vrow[:, :kw], in0=vp_ps[:, :kw],
                                     in1=tha[:, sl])
                nc.sync.dma_start(out=v_dram[sl, 0], in_=vrow[:, :kw])

            # ---- phase C: per-cell records in SBUF ----
            # vm[j] = left-limit of v at cell j's right end (default v[j+1]);
            # vp[j] = right-limit of v at cell j's split (default v[j]).
            nc.sync.dma_start(out=vm_dram[0:C_LOC, 0], in_=v_dram[1 : C_LOC + 1, 0])
            nc.sync.dma_start(out=vp_dram[0:C_LOC, 0], in_=v_dram[0:C_LOC, 0])
            sp2 = wpool.tile([NB, 2], f32, tag="sp2")
            nc.sync.dma_start(out=sp2[:], in_=v_dram[SPEC0 : SPEC0 + 2 * NB, 0])
            nc.gpsimd.indirect_dma_start(
                out=vm_dram[:, :],
                out_offset=IndirectOffsetOnAxis(ap=jl[:, :1], axis=0),
                in_=sp2[:, 0:1], in_offset=None,
            )
            nc.gpsimd.indirect_dma_start(
                out=vp_dram[:, :],
                out_offset=IndirectOffsetOnAxis(ap=jl[:, :1], axis=0),
                in_=sp2[:, 1:2], in_offset=None,
            )
            u_lo = wpool.tile([P, CPP], f32, tag="ulo")
            nc.sync.dma_start(out=u_lo[:], in_=v_dram[0:C_LOC, 0])
            u_hi = wpool.tile([P, CPP], f32, tag="uhi")
            nc.sync.dma_start(out=u_hi[:], in_=v_dram[1 : C_LOC + 1, 0])
            vm = wpool.tile([P, CPP], f32, tag="vm")
            nc.sync.dma_start(out=vm[:], in_=vm_dram[0:C_LOC, 0])
            vpt = wpool.tile([P, CPP], f32, tag="vpt")
            nc.sync.dma_start(out=vpt[:], in_=vp_dram[0:C_LOC, 0])
            b0c = wpool.tile([P, CPP], f32, tag="b0c")
            nc.vector.tensor_sub(out=b0c[:], in0=vm[:], in1=u_lo[:])
            nc.vector.tensor_mul(out=b0c[:], in0=b0c[:], in1=itb[:])
            b1c = wpool.tile([P, CPP], f32, tag="b1c")
            nc.vector.tensor_sub(out=b1c[:], in0=u_hi[:], in1=vpt[:])
            nc.vector.tensor_mul(out=b1c[:], in0=b1c[:], in1=i1tb[:])
            a1c = wpool.tile([P, CPP], f32, tag="a1c")
            nc.vector.tensor_sub(out=a1c[:], in0=u_hi[:], in1=b1c[:])
            da = wpool.tile([P, CPP], f32, tag="da")
            nc.vector.tensor_sub(out=da[:], in0=a1c[:], in1=u_lo[:])
            db = wpool.tile([P, CPP], f32, tag="db")
            nc.vector.tensor_sub(out=db[:], in0=b1c[:], in1=b0c[:])

            # ---- phase D: per-point interpolation ----
            def bcast(tile_, bsl):
                return tile_[:, bsl].to_broadcast([P, PBLK, S])

            for ch in range(NPCH):
                psl = slice(ch * PCH, (ch + 1) * PCH)
                bsl = slice(ch * PBLK, (ch + 1) * PBLK)
                xc = xp[:, psl]
                d = ppool.tile([P, PCH], f32, tag="d")
                nc.vector.tensor_scalar(out=d[:], in0=xc, scalar1=base_c[:],
                                        scalar2=None, op0=Op.subtract)
                s = ppool.tile([P, PCH], f32, tag="s")
                nc.vector.tensor_scalar(out=s[:], in0=d[:], scalar1=float(INVH),
                                        scalar2=c0p[:], op0=Op.mult,
                                        op1=Op.subtract)
                t = ppool.tile([P, PCH], f32, tag="t")
                nc.vector.tensor_sub(out=t[:], in0=s[:], in1=jrow[:, psl])
                x3 = xc.rearrange("p (c s) -> p c s", c=PBLK)
                side = ppool.tile([P, PCH], f32, tag="side")
                s3 = side[:].rearrange("p (c s) -> p c s", c=PBLK)
                nc.vector.tensor_tensor(out=s3, in0=x3, in1=bcast(xs_c, bsl),
                                        op=Op.is_ge)
                # y = (b0 + side*db)*t + (a0 + side*da)
                bb = ppool.tile([P, PCH], f32, tag="bb")
                bb3 = bb[:].rearrange("p (c s) -> p c s", c=PBLK)
                nc.vector.tensor_tensor(out=bb3, in0=s3, in1=bcast(db, bsl),
                                        op=Op.mult)
                nc.vector.tensor_tensor(out=bb3, in0=bb3, in1=bcast(b0c, bsl),
                                        op=Op.add)
                aa = ppool.tile([P, PCH], f32, tag="aa")
                aa3 = aa[:].rearrange("p (c s) -> p c s", c=PBLK)
                nc.vector.tensor_tensor(out=aa3, in0=s3, in1=bcast(da, bsl),
                                        op=Op.mult)
                nc.vector.tensor_tensor(out=aa3, in0=aa3, in1=bcast(u_lo, bsl),
                                        op=Op.add)
                y = ppool.tile([P, PCH], f32, tag="y")
                nc.vector.tensor_mul(out=y[:], in0=bb[:], in1=t[:])
                nc.vector.tensor_add(out=y[:], in0=y[:], in1=aa[:])
                nc.sync.dma_start(out=y_out[:, psl], in_=y[:])

    nc.compile()
    _PROGS[S] = nc
    return nc


# ---------------- host-side input prep ----------------------------------------
def _fold_weights(core, W1, b1, W2, b2, W3, b3):
    means, std, mid, Lb, Rb, bnds = _geometry()
    base = DOM0 + core * DW
    act = [w for w in range(NW) if (Rb[w] > base) and (Lb[w] < base + DW)]
    assert len(act) <= NSLOT, f"core {core}: {len(act)} active windows"
    sc1 = np.zeros((P, NG), np.float32)
    bi1 = np.zeros((P, NG), np.float32)
    w2blk = np.zeros((P, P * NG), np.float32)
    w3f = np.zeros((P, NSLOT * NG), np.float32)
    b2c = np.zeros((P, NG), np.float32)
    b3c = np.zeros((NSLOT, 1), np.float32)
    negl = np.zeros((NSLOT, 1), np.float32)
    rr = np.zeros((NSLOT, 1), np.float32)
    lbc = np.full((NSLOT, 1), BIG, np.float32)
    rbc = np.full((NSLOT, 1), -BIG, np.float32)
    for slot, w in enumerate(act):
        g, s = divmod(slot, 4)
        rows = slice(32 * s, 32 * s + 32)
        w1r = W1[w, 0, :].astype(np.float64)
        sc1[rows, g] = (w1r / std[w]).astype(np.float32)
        bi1[rows, g] = (b1[w] - w1r * means[w] / std[w]).astype(np.float32)
        w2blk[rows, g * P + 32 * s : g * P + 32 * s + 32] = W2[w]
        w3f[rows, g * NSLOT + slot] = W3[w, :, 0]
        b2c[rows, g] = b2[w]
        b3c[slot, 0] = b3[w, 0]
        negl[slot, 0] = -mid[w]
        rr[slot, 0] = mid[w + 1]
        lbc[slot, 0] = np.nextafter(Lb[w], -np.inf)
        rbc[slot, 0] = Rb[w]
    return sc1, bi1, w2blk, w3f, b2c, b3c, negl, rr, lbc, rbc


def _core_tables(core):
    """Knot x-values and straddle-cell helper arrays for one core."""
    means, std, mid, Lb, Rb, bnds = _geometry()
    base = DOM0 + core * DW
    # pad knots equal the last real knot so pad-cell slopes are exactly 0
    knot_row = np.full(KT, np.float32(base + DW), np.float32)
    kidx = np.arange(NKNOT, dtype=np.float64)
    knot_row[:NKNOT] = (base + kidx * H).astype(np.float32)
    bl = [b for b in bnds if base <= b < base + DW]
    assert len(bl) <= NB
    jlist = np.zeros((NB, 1), np.int32)
    xsplit_col = np.full(C_LOC, BIG, np.float32)
    itb_col = np.ones(C_LOC, np.float32)
    i1tb_col = np.ones(C_LOC, np.float32)
    for k, b in enumerate(bl):
        bf = np.float32(b)
        j = int(np.floor((float(bf) - base) / H))
        assert 0 <= j < C_LOC
        tB = (float(bf) - (base + j * H)) / H
        tB = min(max(tB, 1e-7), 1 - 1e-7)
        jlist[k, 0] = j
        xsplit_col[j] = bf
        itb_col[j] = np.float32(1.0 / tB)
        i1tb_col[j] = np.float32(1.0 / (1.0 - tB))
        knot_row[SPEC0 + 2 * k] = np.nextafter(bf, np.float32(-np.inf))
        knot_row[SPEC0 + 2 * k + 1] = bf
    for k in range(len(bl), NB):
        jlist[k, 0] = C_LOC + k       # dummy scatter rows, never read back
    knotrep = np.broadcast_to(knot_row, (P, KT)).copy()
    knot12 = np.broadcast_to(knot_row, (NSLOT, KT)).copy()
    # window mask at every knot: (knot > nextbelow(Lb)) & (knot < Rb) per slot
    base2 = DOM0 + core * DW
    act = [w for w in range(NW) if (Rb[w] > base2) and (Lb[w] < base2 + DW)]
    wmask = np.zeros((NSLOT, KT), np.float32)
    for slot, w in enumerate(act):
        lbv = np.nextafter(Lb[w], -np.inf)
        wmask[slot] = ((knot_row > lbv) & (knot_row < Rb[w])).astype(np.float32)
    return (knotrep, knot12, xsplit_col.reshape(P, CPP), itb_col.reshape(P, CPP),
            i1tb_col.reshape(P, CPP), jlist, wmask)


def _prep_in_maps(inputs, S):
    x = np.asarray(inputs["x"], np.float32)
    W1 = np.asarray(inputs["W1"], np.float32)
    b1 = np.asarray(inputs["b1"], np.float32)
    W2 = np.asarray(inputs["W2"], np.float32)
    b2 = np.asarray(inputs["b2"], np.float32)
    W3 = np.asarray(inputs["W3"], np.float32)
    b3 = np.asarray(inputs["b3"], np.float32)
    M = CPP * S

    # global cell of each point, then slot position inside the padded grid
    cglob = np.minimum((x.astype(np.float64) * (1.0 / H)).astype(np.int64),
                       NCORES * C_LOC - 1)
    order = np.argsort(cglob, kind="stable")
    cs = cglob[order]
    cnt = np.bincount(cglob, minlength=NCORES * C_LOC)
    maxcnt = int(cnt.max())
    if maxcnt > S:
        raise OverflowError(maxcnt)
    starts = np.concatenate(([0], np.cumsum(cnt)))
    rank = np.arange(len(x)) - starts[cs]           # rank within own cell
    slot = cs * S + rank                            # global padded slot index

    in_maps = []
    for core in range(NCORES):
        base = np.float32(DOM0 + core * DW)
        # pad x with each cell's left-edge x so t~0 and y=a0 (finite, discarded)
        cellx = (base + np.arange(C_LOC, dtype=np.float64) * H).astype(np.float32)
        xpad = np.repeat(cellx, S)
        msk = (cs >= core * C_LOC) & (cs < (core + 1) * C_LOC)
        xpad[slot[msk] - core * C_LOC * S] = x[order[msk]]
        sc1, bi1, w2blk, w3f, b2c, b3c, negl, rr, lbc, rbc = _fold_weights(
            core, W1, b1, W2, b2, W3, b3)
        (knotrep, knot12, xsplit_col, itb_col, i1tb_col, jlist,
         wmask) = _core_tables(core)
        in_maps.append({
            "x_pts": xpad.reshape(P, M),
            "base_col": np.full((P, 1), base, np.float32),
            "c0p_col": (np.arange(P, dtype=np.float32) * CPP).reshape(P, 1),
            "sc1": sc1, "bi1": bi1, "w2blk": w2blk, "w3f": w3f,
            "b2c": b2c, "b3c": b3c, "negl": negl, "rr": rr,
            "lbnd": lbc, "rbnd": rbc,
            "knotrep": knotrep, "knot12": knot12, "xsplit_col": xsplit_col,
            "invtb_col": itb_col, "inv1mtb_col": i1tb_col,
            "jlist": jlist, "wmask": wmask,
            "ones12": np.ones((NSLOT, 1), np.float32),
        })
    return in_maps, order, slot


def _unpack(results, order, slot, n_total):
    allys = np.concatenate([r["y_out"].reshape(-1) for r in results])
    out = np.empty(n_total, np.float32)
    out[order] = allys[slot]
    return out


def kernel(**inputs) -> np.ndarray:
    from concourse.bass_utils import run_bass_kernel_spmd

    S = S_DEFAULT
    while True:
        try:
            in_maps, order, slot = _prep_in_maps(inputs, S)
            break
        except OverflowError as e:
            S = ((int(e.args[0]) + 11) // 8) * 8   # headroom, multiple of 8
    nc = _build_program(S)
    res = run_bass_kernel_spmd(nc, in_maps, list(range(NCORES)))
    return _unpack(res.results, order, slot, len(np.asarray(inputs["x"])))



# revision 11
# speedup vs baseline: 3.6425x; 3.6425x over previous
"""FBPinn forward kernel for Trainium2 (8 NeuronCores, Bass/Tile).

y(x) = tanh(x) * sum_w [win_w(x)>1e-3] * win_w(x) * MLP_w(x) for 1M points.
v2 design (vs the 4096-cell baseline): a 128-cell-per-core grid (cell =
partition), tolerable because piecewise-linear error scales h^2 and the
measured rel-err at this resolution is 2e-3 vs the 2e-2 gate.

Per core:
  B. evaluate the function at the 129 knots + 2*NB straddle-limit x's of the
     core's 12.5-wide slice: 3 block-diag matmuls (f32r fast path) + tanh on
     ACT; window sigmoids via tanh (one ACT table); tanh(x)*mask folded into a
     host constant.
  C. knot row -> per-cell records entirely on-chip: the slot-reduce matmul
     (lhsT=full, rhs=ones) yields [128,1] columns directly; straddle-cell
     two-sided limits are scattered by constant one-hot select matmuls.
     Records (a0,b0,a1,b1,tB) are per-partition scalars (CPP=1).
  D. interpolation as fused tensor_scalar/ACT-Copy ops: y0 = Copy(t*b0+a0) on
     ACT, y1 = t*b1+a1 on gpsimd, side = (t>=tB) and copy_predicated on DVE.
Host shards points by domain, packs each point's fractional cell coordinate t
into a (cell=partition, slot) grid, and un-permutes the outputs.
"""

import numpy as np

# ---------------- problem constants (hardcoded from the module spec) ----------
NW = 30
DOM0, DOM1 = 0.0, 100.0
OVERLAP = 0.25
NEURONS = 32
THRESH = 0.001
N = 1_000_000

NCORES = 8
P = 128                      # SBUF partitions == cells per core
C_LOC = P                    # cells per core
DW = 12.5                    # per-core domain width
H = DW / C_LOC               # global cell width (25*2^-8, exact in fp32)
NG = 3                       # window groups of 4 per core
NSLOT = 4 * NG               # window slots per core
NKNOT = C_LOC + 1            # real grid knots per core
NB = 16                      # straddle-boundary slots per core
SPEC0 = 136                  # first special knot column
KT = SPEC0 + 2 * NB          # knot columns (129 real + pad + 32 specials)
S_DEFAULT = 1104             # point slots per cell
NCH_D = 2                    # phase-D chunks
BIG = np.float32(1e30)


# ---------------- geometry (host, input-independent) --------------------------
def _partition_geom():
    width = (DOM1 - DOM0) / NW
    sub = np.zeros((NW, 2), np.float32)
    for i in range(NW):
        sub[i, 0] = DOM0 if i == 0 else DOM0 + (i - OVERLAP / 2) * width
        sub[i, 1] = DOM1 if i == NW - 1 else DOM0 + (i + 1 + OVERLAP / 2) * width
    means = (sub[:, 0] + sub[:, 1]) / 2
    std = (sub[:, 1] - sub[:, 0]) / 2
    mid = np.zeros(NW + 1, np.float32)
    mid[0] = sub[0, 0]
    mid[-1] = sub[-1, 1]
    for i in range(1, NW):
        mid[i] = (sub[i - 1, 1] + sub[i, 0]) / 2
    return means.astype(np.float32), std.astype(np.float32), mid.astype(np.float32)


def _win64(l, r, x):
    return 1.0 / (1 + np.exp(-(x - l))) / (1 + np.exp(x - r))


def _bisect64(l, r, lo, hi, rising):
    for _ in range(200):
        m = 0.5 * (lo + hi)
        if (_win64(l, r, m) < THRESH) == rising:
            lo = m
        else:
            hi = m
    return 0.5 * (lo + hi)


def _refine_flip_fp32(l32, r32, b64, rising):
    """Exact fp32 x where the reference's jax-fp32 predicate win(x)>1e-3 flips.
    Returns the smallest fp32 x at which the predicate equals its right-side
    state. Falls back to the float64 bisection value if jax is unavailable."""
    try:
        import jax
        import jax.numpy as jnp

        cpu = jax.devices("cpu")[0]
        lo = np.float32(b64 - 5e-5)
        hi = np.float32(b64 + 5e-5)
        xs = np.arange(lo.view(np.int32), hi.view(np.int32) + 1,
                       dtype=np.int32).view(np.float32)
        with jax.default_device(cpu):
            win = np.asarray(
                jax.nn.sigmoid(jnp.asarray(xs) - np.float32(l32))
                * jax.nn.sigmoid(-(jnp.asarray(xs) - np.float32(r32)))
            )
        pred = win > np.float32(THRESH)
        state = pred if rising else ~pred
        if not state.any() or state.all():
            return np.float32(b64)
        k = int(np.argmax(state))
        if not state[k:].all():
            return np.float32(b64)
        return xs[k]
    except Exception:
        return np.float32(b64)


_GEOM = None


def _geometry():
    global _GEOM
    if _GEOM is not None:
        return _GEOM
    means, std, mid = _partition_geom()
    ml = mid[:-1].astype(np.float64)
    mr = mid[1:].astype(np.float64)
    Lb = np.zeros(NW, np.float32)   # window-on lower bound (exact fp32 flip)
    Rb = np.zeros(NW, np.float32)   # window-off upper bound
    for w in range(NW):
        c = 0.5 * (ml[w] + mr[w])
        l64 = _bisect64(ml[w], mr[w], ml[w] - 30, c, rising=True)
        r64 = _bisect64(ml[w], mr[w], c, mr[w] + 30, rising=False)
        Lb[w] = _refine_flip_fp32(mid[w], mid[w + 1], l64, rising=True)
        Rb[w] = _refine_flip_fp32(mid[w], mid[w + 1], r64, rising=False)
    bnds = []
    for w in range(NW):
        if DOM0 < Lb[w] < DOM1:
            bnds.append(float(Lb[w]))
        if DOM0 < Rb[w] < DOM1:
            bnds.append(float(Rb[w]))
    bnds = np.sort(np.array(bnds, np.float64))
    _GEOM = (means, std, mid, Lb, Rb, bnds)
    return _GEOM


# ---------------- bass program (built once per S, SPMD across 8 cores) --------
_PROGS = {}


def _build_program(S):
    if S in _PROGS:
        return _PROGS[S]
    from concourse import bacc, mybir, tile

    f32 = mybir.dt.float32
    f32r = mybir.dt.float32r
    u8 = mybir.dt.uint8
    Act = mybir.ActivationFunctionType
    Op = mybir.AluOpType

    CHW = S // NCH_D

    nc = bacc.Bacc(None, target_bir_lowering=False)

    t_in = nc.declare_dram_parameter("t_pts", [P, S], f32, isOutput=False)
    k24_in = nc.declare_dram_parameter("k24sb", [64, KT + 2], f32, isOutput=False)
    kr_in = nc.declare_dram_parameter("knotrep", [P, KT], f32, isOutput=False)
    pc_in = nc.declare_dram_parameter("pconst", [P, 13], f32, isOutput=False)
    w2_in = nc.declare_dram_parameter("w2blk", [P, P * NG], f32r, isOutput=False)
    w3_in = nc.declare_dram_parameter("w3f", [P, NSLOT * NG], f32r, isOutput=False)
    b3_in = nc.declare_dram_parameter("b3c", [NSLOT, 1], f32, isOutput=False)
    wm_in = nc.declare_dram_parameter("wmaskp", [NSLOT, KT], f32, isOutput=False)
    um_in = nc.declare_dram_parameter("UMP", [2 * NB, 2 * P], f32, isOutput=False)
    y_out = nc.declare_dram_parameter("y_out", [P, S], f32, isOutput=True)

    with tile.TileContext(nc) as tc:
        with (
            tc.tile_pool(name="const", bufs=1) as cpool,
            tc.tile_pool(name="work", bufs=2) as wpool,
            tc.tile_pool(name="psum", bufs=1, space="PSUM") as psum,
        ):
            # ---- constant loads ----
            tp = cpool.tile([P, S], f32, tag="c_t")
            nc.gpsimd.dma_start(out=tp[:], in_=t_in[:])
            knots = cpool.tile([P, KT], f32, tag="c_kr")
            nc.gpsimd.dma_start(out=knots[:], in_=kr_in[:])
            k24 = cpool.tile([64, KT + 2], f32, tag="c_k24")
            nc.gpsimd.dma_start(out=k24[:], in_=k24_in[:])
            pconst = cpool.tile([P, 13], f32, tag="c_pc")
            nc.gpsimd.dma_start(out=pconst[:], in_=pc_in[:])
            w2 = cpool.tile([P, P * NG], f32r, tag="c_w2")
            nc.scalar.dma_start(out=w2[:], in_=w2_in[:])
            w3 = cpool.tile([P, NSLOT * NG], f32r, tag="c_w3")
            nc.sync.dma_start(out=w3[:], in_=w3_in[:])
            wmaskp = cpool.tile([NSLOT, KT], f32, tag="c_wm")
            nc.sync.dma_start(out=wmaskp[:], in_=wm_in[:])
            ump = cpool.tile([2 * NB, 2 * P], f32, tag="c_um")
            nc.sync.dma_start(out=ump[:], in_=um_in[:])
            b3c = cpool.tile([NSLOT, 1], f32, tag="c_b3")
            nc.sync.dma_start(out=b3c[:], in_=b3_in[:])
            ones12 = cpool.tile([NSLOT, 1], f32, tag="c_o12")
            nc.gpsimd.memset(ones12[:], 1.0)

            sc1 = pconst[:, 0:NG]
            bi1 = pconst[:, NG:2 * NG]
            b2c = pconst[:, 2 * NG:3 * NG]
            selc = pconst[:, 9:10]
            itb = pconst[:, 10:11]
            i1tb = pconst[:, 11:12]
            tbc = pconst[:, 12:13]

            # ---- phase B: knot values ----
            # win-pair tanh rows (sigmoid(z) = 0.5*(1+tanh(z/2)): one table)
            w24 = cpool.tile([64, KT], f32, tag="w24")
            h1 = [cpool.tile([P, KT], f32r, tag=f"h1_{g}", name=f"h1_{g}")
                  for g in range(NG)]
            h2 = [cpool.tile([P, KT], f32r, tag=f"h2_{g}", name=f"h2_{g}")
                  for g in range(NG)]
            nc.scalar.activation(out=h1[0][:], in_=knots[:], func=Act.Tanh,
                                 bias=bi1[:, 0:1], scale=sc1[:, 0:1])
            nc.scalar.activation(out=w24[:], in_=k24[:, 0:KT], func=Act.Tanh,
                                 bias=k24[:, KT + 1:KT + 2],
                                 scale=k24[:, KT:KT + 1])
            nc.scalar.activation(out=h1[1][:], in_=knots[:], func=Act.Tanh,
                                 bias=bi1[:, 1:2], scale=sc1[:, 1:2])

            h2ps = psum.tile([P, NG * KT], f32, tag="h2ps")
            nc.tensor.matmul(out=h2ps[:, 0:KT], lhsT=w2[:, 0:P], rhs=h1[0][:],
                             start=True, stop=True)
            nc.scalar.activation(out=h2[0][:], in_=h2ps[:, 0:KT],
                                 func=Act.Tanh, bias=b2c[:, 0:1], scale=1.0)
            nc.scalar.activation(out=h1[2][:], in_=knots[:], func=Act.Tanh,
                                 bias=bi1[:, 2:3], scale=sc1[:, 2:3])
            nc.tensor.matmul(out=h2ps[:, KT:2 * KT], lhsT=w2[:, P:2 * P],
                             rhs=h1[1][:], start=True, stop=True)
            nc.scalar.activation(out=h2[1][:], in_=h2ps[:, KT:2 * KT],
                                 func=Act.Tanh, bias=b2c[:, 1:2], scale=1.0)
            nc.tensor.matmul(out=h2ps[:, 2 * KT:3 * KT], lhsT=w2[:, 2 * P:3 * P],
                             rhs=h1[2][:], start=True, stop=True)
            nc.scalar.activation(out=h2[2][:], in_=h2ps[:, 2 * KT:3 * KT],
                                 func=Act.Tanh, bias=b2c[:, 2:3], scale=1.0)

            pre = psum.tile([NSLOT, KT], f32, tag="pre")
            for g in range(NG):
                nc.tensor.matmul(out=pre[:],
                                 lhsT=w3[:, g * NSLOT:(g + 1) * NSLOT],
                                 rhs=h2[g][:], start=(g == 0), stop=(g == 2))

            # window = 0.25*(1+t1)*(1+t2), then *wmaskp (mask * tanh(knot))
            ws1 = wpool.tile([NSLOT, KT], f32, tag="ws1")
            nc.gpsimd.tensor_scalar(out=ws1[:], in0=w24[0:NSLOT, :],
                                    scalar1=1.0, scalar2=0.25,
                                    op0=Op.add, op1=Op.mult)
            ws2 = wpool.tile([NSLOT, KT], f32, tag="ws2")
            nc.gpsimd.tensor_scalar(out=ws2[:], in0=w24[32:32 + NSLOT, :],
                                    scalar1=1.0, scalar2=None, op0=Op.add)
            win = wpool.tile([NSLOT, KT], f32, tag="win")
            nc.vector.tensor_mul(out=win[:], in0=ws1[:], in1=ws2[:])
            nc.vector.tensor_mul(out=win[:], in0=win[:], in1=wmaskp[:])
            term = wpool.tile([NSLOT, KT], f32, tag="term")
            nc.vector.tensor_scalar(out=term[:], in0=pre[:], scalar1=b3c[:],
                                    scalar2=None, op0=Op.add)
            full = wpool.tile([NSLOT, KT], f32, tag="full")
            nc.vector.tensor_mul(out=full[:], in0=term[:], in1=win[:])

            # ---- phase C: records ([128,1] per-partition scalars) ----
            redu = psum.tile([P, 8], f32, tag="redu")
            u_loP = redu[:, 0:1]
            u_hiP = redu[:, 1:2]
            spP = redu[0:2 * NB, 2:3]
            vmP = redu[:, 3:4]
            vpP = redu[:, 4:5]
            nc.tensor.matmul(out=u_loP, lhsT=full[:, 0:P], rhs=ones12[:],
                             start=True, stop=True)
            nc.tensor.matmul(out=u_hiP, lhsT=full[:, 1:P + 1], rhs=ones12[:],
                             start=True, stop=True)
            nc.tensor.matmul(out=spP, lhsT=full[:, SPEC0:SPEC0 + 2 * NB],
                             rhs=ones12[:], start=True, stop=True)
            spS = wpool.tile([2 * NB, 1], f32, tag="spS")
            nc.vector.tensor_copy(out=spS[:], in_=spP)
            nc.tensor.matmul(out=vmP, lhsT=ump[:, 0:P], rhs=spS[:],
                             start=True, stop=True)
            nc.tensor.matmul(out=vpP, lhsT=ump[:, P:2 * P], rhs=spS[:],
                             start=True, stop=True)

            u_lo = cpool.tile([P, 1], f32, tag="ulo")
            nc.vector.tensor_copy(out=u_lo[:], in_=u_loP)
            u_hi = cpool.tile([P, 1], f32, tag="uhi")
            nc.vector.tensor_copy(out=u_hi[:], in_=u_hiP)
            vm = wpool.tile([P, 1], f32, tag="vm")
            nc.vector.tensor_scalar(out=vm[:], in0=u_hi[:], scalar1=selc,
                                    scalar2=None, op0=Op.mult)
            nc.vector.tensor_add(out=vm[:], in0=vm[:], in1=vmP)
            vp = wpool.tile([P, 1], f32, tag="vp")
            nc.vector.tensor_scalar(out=vp[:], in0=u_lo[:], scalar1=selc,
                                    scalar2=None, op0=Op.mult)
            nc.vector.tensor_add(out=vp[:], in0=vp[:], in1=vpP)
            b0 = cpool.tile([P, 1], f32, tag="b0")
            nc.vector.tensor_sub(out=b0[:], in0=vm[:], in1=u_lo[:])
            nc.vector.tensor_scalar(out=b0[:], in0=b0[:], scalar1=itb,
                                    scalar2=None, op0=Op.mult)
            b1 = cpool.tile([P, 1], f32, tag="b1")
            nc.vector.tensor_sub(out=b1[:], in0=u_hi[:], in1=vp[:])
            nc.vector.tensor_scalar(out=b1[:], in0=b1[:], scalar1=i1tb,
                                    scalar2=None, op0=Op.mult)
            a1 = cpool.tile([P, 1], f32, tag="a1")
            nc.vector.tensor_sub(out=a1[:], in0=u_hi[:], in1=b1[:])

            # ---- phase D: per-point interpolation ----
            for ch in range(NCH_D):
                sl = slice(ch * CHW, (ch + 1) * CHW)
                ybuf = wpool.tile([P, CHW], f32, tag="ybuf")
                nc.scalar.activation(out=ybuf[:], in_=tp[:, sl],
                                     func=Act.Identity,
                                     bias=u_lo[:], scale=b0[:])
                y1 = wpool.tile([P, CHW], f32, tag="y1")
                nc.gpsimd.tensor_scalar(out=y1[:], in0=tp[:, sl],
                                        scalar1=b1[:], scalar2=a1[:],
                                        op0=Op.mult, op1=Op.add)
                side = wpool.tile([P, CHW], u8, tag="side")
                nc.vector.tensor_scalar(out=side[:], in0=tp[:, sl],
                                        scalar1=tbc, scalar2=None,
                                        op0=Op.is_ge)
                nc.vector.copy_predicated(out=ybuf[:], mask=side[:], data=y1[:])
                nc.gpsimd.dma_start(out=y_out[:, sl], in_=ybuf[:])

    nc.compile()
    _PROGS[S] = nc
    return nc


# ---------------- host-side input prep ----------------------------------------
def _fold_weights(core, W1, b1, W2, b2, W3, b3):
    means, std, mid, Lb, Rb, bnds = _geometry()
    base = DOM0 + core * DW
    act = [w for w in range(NW) if (Rb[w] > base) and (Lb[w] < base + DW)]
    assert len(act) <= NSLOT, f"core {core}: {len(act)} active windows"
    sc1 = np.zeros((P, NG), np.float32)
    bi1 = np.zeros((P, NG), np.float32)
    w2blk = np.zeros((P, P * NG), np.float32)
    w3f = np.zeros((P, NSLOT * NG), np.float32)
    b2c = np.zeros((P, NG), np.float32)
    b3c = np.zeros((NSLOT, 1), np.float32)
    winsc = np.zeros((64, 1), np.float32)
    winbi = np.zeros((64, 1), np.float32)
    for slot, w in enumerate(act):
        g, s = divmod(slot, 4)
        rows = slice(32 * s, 32 * s + 32)
        w1r = W1[w, 0, :].astype(np.float64)
        sc1[rows, g] = (w1r / std[w]).astype(np.float32)
        bi1[rows, g] = (b1[w] - w1r * means[w] / std[w]).astype(np.float32)
        w2blk[rows, g * P + 32 * s: g * P + 32 * s + 32] = W2[w]
        w3f[rows, g * NSLOT + slot] = W3[w, :, 0]
        b2c[rows, g] = b2[w]
        b3c[slot, 0] = b3[w, 0]
        winsc[slot, 0] = 0.5
        winbi[slot, 0] = np.float32(-0.5 * float(mid[w]))
        winsc[32 + slot, 0] = -0.5
        winbi[32 + slot, 0] = np.float32(0.5 * float(mid[w + 1]))
    return sc1, bi1, w2blk, w3f, b2c, b3c, winsc, winbi, act


_TABLES = None


def _core_tables(core):
    """Input-independent per-core constant tables (cached)."""
    global _TABLES
    if _TABLES is None:
        _TABLES = {}
    if core in _TABLES:
        return _TABLES[core]
    means, std, mid, Lb, Rb, bnds = _geometry()
    base = DOM0 + core * DW
    knot_row = np.full(KT, np.float32(base + DW), np.float32)
    kidx = np.arange(NKNOT, dtype=np.float64)
    knot_row[:NKNOT] = (base + kidx * H).astype(np.float32)
    bl = [b for b in bnds if base <= b < base + DW]
    assert len(bl) <= NB
    ump = np.zeros((2 * NB, 2 * P), np.float32)
    selc = np.ones((P, 1), np.float32)
    itb_col = np.ones((P, 1), np.float32)
    i1tb_col = np.ones((P, 1), np.float32)
    tb_col = np.full((P, 1), 2.0, np.float32)
    for k, b in enumerate(bl):
        bf = np.float32(b)
        g64 = float(bf) / H
        jg = int(np.floor(g64))
        j = jg - core * C_LOC
        assert 0 <= j < C_LOC
        tB = g64 - jg
        tB = min(max(tB, 1e-7), 1 - 1e-7)
        selc[j, 0] = 0.0
        itb_col[j, 0] = np.float32(1.0 / tB)
        i1tb_col[j, 0] = np.float32(1.0 / (1.0 - tB))
        tb_col[j, 0] = np.float32(tB)
        knot_row[SPEC0 + 2 * k] = np.nextafter(bf, np.float32(-np.inf))
        knot_row[SPEC0 + 2 * k + 1] = bf
        ump[2 * k, j] = 1.0          # vm <- left-limit special
        ump[2 * k + 1, P + j] = 1.0  # vp <- right-limit special
    knotrep = np.broadcast_to(knot_row, (P, KT)).copy()
    k24row = np.broadcast_to(knot_row, (64, KT)).copy()
    # wmaskp = window mask at knots * tanh(knot): the final ansatz folded in
    act = [w for w in range(NW) if (Rb[w] > base) and (Lb[w] < base + DW)]
    th = np.tanh(knot_row.astype(np.float64))
    wmaskp = np.zeros((NSLOT, KT), np.float32)
    for slot, w in enumerate(act):
        lbv = np.nextafter(Lb[w], -np.inf)
        m = (knot_row > lbv) & (knot_row < Rb[w])
        wmaskp[slot] = (m * th).astype(np.float32)
    out = (knotrep, k24row, wmaskp, ump, selc, itb_col, i1tb_col, tb_col)
    _TABLES[core] = out
    return out


def _prep_in_maps(inputs, S):
    x = np.asarray(inputs["x"], np.float32)
    W1 = np.asarray(inputs["W1"], np.float32)
    b1 = np.asarray(inputs["b1"], np.float32)
    W2 = np.asarray(inputs["W2"], np.float32)
    b2 = np.asarray(inputs["b2"], np.float32)
    W3 = np.asarray(inputs["W3"], np.float32)
    b3 = np.asarray(inputs["b3"], np.float32)

    g64 = x.astype(np.float64) / H
    cglob = np.minimum(g64.astype(np.int64), NCORES * C_LOC - 1)
    tfrac = (g64 - cglob).astype(np.float32)
    order = np.argsort(cglob, kind="stable")
    cs = cglob[order]
    cnt = np.bincount(cglob, minlength=NCORES * C_LOC)
    maxcnt = int(cnt.max())
    if maxcnt > S:
        raise OverflowError(maxcnt)
    starts = np.concatenate(([0], np.cumsum(cnt)))
    rank = np.arange(len(x)) - starts[cs]           # rank within own cell
    slot = cs * S + rank                            # global padded slot index

    in_maps = []
    for core in range(NCORES):
        tpad = np.zeros(C_LOC * S, np.float32)      # pad t=0 -> y=a0 (finite)
        msk = (cs >= core * C_LOC) & (cs < (core + 1) * C_LOC)
        tpad[slot[msk] - core * C_LOC * S] = tfrac[order[msk]]
        sc1, bi1, w2blk, w3f, b2c, b3c, winsc, winbi, _ = _fold_weights(
            core, W1, b1, W2, b2, W3, b3)
        (knotrep, k24row, wmaskp, ump, selc, itb_col, i1tb_col,
         tb_col) = _core_tables(core)
        pconst = np.concatenate([sc1, bi1, b2c, selc, itb_col, i1tb_col,
                                 tb_col], axis=1)
        k24sb = np.concatenate([k24row, winsc, winbi], axis=1)
        in_maps.append({
            "t_pts": tpad.reshape(P, S),
            "k24sb": k24sb,
            "knotrep": knotrep,
            "pconst": pconst,
            "w2blk": w2blk,
            "w3f": w3f,
            "b3c": b3c,
            "wmaskp": wmaskp,
            "UMP": ump,
        })
    return in_maps, order, slot


def _unpack(results, order, slot, n_total):
    allys = np.concatenate([r["y_out"].reshape(-1) for r in results])
    out = np.empty(n_total, np.float32)
    out[order] = allys[slot]
    return out


def kernel(**inputs) -> np.ndarray:
    from concourse.bass_utils import run_bass_kernel_spmd

    S = S_DEFAULT
    while True:
        try:
            in_maps, order, slot = _prep_in_maps(inputs, S)
            break
        except OverflowError as e:
            S = ((int(e.args[0]) + 2 * NCH_D - 1) // (2 * NCH_D)) * (2 * NCH_D)
    nc = _build_program(S)
    res = run_bass_kernel_spmd(nc, in_maps, list(range(NCORES)))
    return _unpack(res.results, order, slot, len(np.asarray(inputs["x"])))


# revision 14
# speedup vs baseline: 3.9642x; 1.0883x over previous
"""FBPinn forward kernel for Trainium2 (8 NeuronCores, Bass/Tile).

y(x) = tanh(x) * sum_w [win_w(x)>1e-3] * win_w(x) * MLP_w(x) for 1M points.
Strategy: tabulate the scalar function on a coarse uniform grid (PL error
~2e-3 vs the 2e-2 gate) and interpolate; all discontinuity handling is
resolved on the host.

Layout: 120 grid cells + 8 spare partitions per core (cell = partition).
Straddle cells (window-mask flips inside the cell) keep their left segment;
right-segment points are repacked to a spare partition. The host sends each
point's segment-normalized coordinate tau in [0,1), so every partition's
answer is y = a + b*tau with per-partition scalars a, b:
  B. evaluate the function at 256 knot columns (120 knots | 8 right-limits |
     120 shifted knots | 8 left-limits) via 3 block-diag f32r matmuls + tanh
     on ACT; window sigmoids via tanh (single ACT table); window mask and the
     tanh(x) ansatz folded into one host constant.
  C. records fully on-chip: slot-reduce matmuls give the a-column and the
     hi-column directly ([128,1]); one constant permutation matmul swaps
     (left-limits -> straddle cells, cell hi -> spares); b = hi' - a.
  D. per chunk, one fused op: tensor_scalar(t*b+a) on DVE / Identity ACT on
     Scalar. No compares, no selects.
"""

import numpy as np

# ---------------- problem constants (hardcoded from the module spec) ----------
NW = 30
DOM0, DOM1 = 0.0, 100.0
OVERLAP = 0.25
NEURONS = 32
THRESH = 0.001
N = 1_000_000

NCORES = 8
P = 128                      # SBUF partitions
CL = 120                     # grid cells per core (partitions 120..127 spare)
DW = 12.5                    # per-core domain width
H = DW / CL                  # global cell width
NG = 3                       # window groups of 4 per core
NSLOT = 4 * NG               # window slots per core
NB = 8                       # straddle-boundary slots per core
KT = 256                     # knot columns
S_DEFAULT = 1168             # point slots per partition
NCH_D = 4                    # phase-D chunks


# ---------------- geometry (host, input-independent) --------------------------
def _partition_geom():
    width = (DOM1 - DOM0) / NW
    sub = np.zeros((NW, 2), np.float32)
    for i in range(NW):
        sub[i, 0] = DOM0 if i == 0 else DOM0 + (i - OVERLAP / 2) * width
        sub[i, 1] = DOM1 if i == NW - 1 else DOM0 + (i + 1 + OVERLAP / 2) * width
    means = (sub[:, 0] + sub[:, 1]) / 2
    std = (sub[:, 1] - sub[:, 0]) / 2
    mid = np.zeros(NW + 1, np.float32)
    mid[0] = sub[0, 0]
    mid[-1] = sub[-1, 1]
    for i in range(1, NW):
        mid[i] = (sub[i - 1, 1] + sub[i, 0]) / 2
    return means.astype(np.float32), std.astype(np.float32), mid.astype(np.float32)


def _win64(l, r, x):
    return 1.0 / (1 + np.exp(-(x - l))) / (1 + np.exp(x - r))


def _bisect64(l, r, lo, hi, rising):
    for _ in range(200):
        m = 0.5 * (lo + hi)
        if (_win64(l, r, m) < THRESH) == rising:
            lo = m
        else:
            hi = m
    return 0.5 * (lo + hi)


def _refine_flip_fp32(l32, r32, b64, rising):
    """Exact fp32 x where the reference's jax-fp32 predicate win(x)>1e-3 flips.
    Returns the smallest fp32 x at which the predicate equals its right-side
    state. Falls back to the float64 bisection value if jax is unavailable."""
    try:
        import jax
        import jax.numpy as jnp

        cpu = jax.devices("cpu")[0]
        lo = np.float32(b64 - 5e-5)
        hi = np.float32(b64 + 5e-5)
        xs = np.arange(lo.view(np.int32), hi.view(np.int32) + 1,
                       dtype=np.int32).view(np.float32)
        with jax.default_device(cpu):
            win = np.asarray(
                jax.nn.sigmoid(jnp.asarray(xs) - np.float32(l32))
                * jax.nn.sigmoid(-(jnp.asarray(xs) - np.float32(r32)))
            )
        pred = win > np.float32(THRESH)
        state = pred if rising else ~pred
        if not state.any() or state.all():
            return np.float32(b64)
        k = int(np.argmax(state))
        if not state[k:].all():
            return np.float32(b64)
        return xs[k]
    except Exception:
        return np.float32(b64)


_GEOM = None


def _geometry():
    global _GEOM
    if _GEOM is not None:
        return _GEOM
    means, std, mid = _partition_geom()
    ml = mid[:-1].astype(np.float64)
    mr = mid[1:].astype(np.float64)
    Lb = np.zeros(NW, np.float32)   # window-on lower bound (exact fp32 flip)
    Rb = np.zeros(NW, np.float32)   # window-off upper bound
    for w in range(NW):
        c = 0.5 * (ml[w] + mr[w])
        l64 = _bisect64(ml[w], mr[w], ml[w] - 30, c, rising=True)
        r64 = _bisect64(ml[w], mr[w], c, mr[w] + 30, rising=False)
        Lb[w] = _refine_flip_fp32(mid[w], mid[w + 1], l64, rising=True)
        Rb[w] = _refine_flip_fp32(mid[w], mid[w + 1], r64, rising=False)
    bnds = []
    for w in range(NW):
        if DOM0 < Lb[w] < DOM1:
            bnds.append(float(Lb[w]))
        if DOM0 < Rb[w] < DOM1:
            bnds.append(float(Rb[w]))
    bnds = np.sort(np.array(bnds, np.float64))
    _GEOM = (means, std, mid, Lb, Rb, bnds)
    return _GEOM


# ---------------- bass program (built once per S, SPMD across 8 cores) --------
_PROGS = {}


def _build_program(S):
    if S in _PROGS:
        return _PROGS[S]
    from concourse import bacc, mybir, tile

    f32 = mybir.dt.float32
    f32r = mybir.dt.float32r
    Act = mybir.ActivationFunctionType
    Op = mybir.AluOpType

    CHW = S // NCH_D

    nc = bacc.Bacc(None, target_bir_lowering=False)

    t_in = nc.declare_dram_parameter("t_pts", [P, S], f32, isOutput=False)
    k24_in = nc.declare_dram_parameter("k24sb", [64, KT + 2], f32, isOutput=False)
    kr_in = nc.declare_dram_parameter("knotrep", [P, KT], f32, isOutput=False)
    pc_in = nc.declare_dram_parameter("pconst", [P, 10], f32, isOutput=False)
    w2_in = nc.declare_dram_parameter("w2blk", [P, P * NG], f32r, isOutput=False)
    w3_in = nc.declare_dram_parameter("w3f", [P, NSLOT * NG], f32r, isOutput=False)
    b3_in = nc.declare_dram_parameter("b3c", [NSLOT, 1], f32, isOutput=False)
    wm_in = nc.declare_dram_parameter("wmaskp", [NSLOT, KT], f32, isOutput=False)
    pm_in = nc.declare_dram_parameter("permM", [P, P], f32r, isOutput=False)
    on_in = nc.declare_dram_parameter("ones2r", [NSLOT, 2], f32r, isOutput=False)
    y_out = nc.declare_dram_parameter("y_out", [P, S], f32, isOutput=True)

    with tile.TileContext(nc) as tc:
        with (
            tc.tile_pool(name="const", bufs=1) as cpool,
            tc.tile_pool(name="work", bufs=2) as wpool,
            tc.tile_pool(name="psum", bufs=1, space="PSUM") as psum,
        ):
            # ---- constant loads (small critical tables first per queue) ----
            knots = cpool.tile([P, KT], f32, tag="c_kr")
            nc.gpsimd.dma_start(out=knots[:], in_=kr_in[:])
            k24 = cpool.tile([64, KT + 2], f32, tag="c_k24")
            nc.gpsimd.dma_start(out=k24[:], in_=k24_in[:])
            pconst = cpool.tile([P, 10], f32, tag="c_pc")
            nc.gpsimd.dma_start(out=pconst[:], in_=pc_in[:])
            tp = cpool.tile([P, S], f32, tag="c_t")
            nc.gpsimd.dma_start(out=tp[:], in_=t_in[:])
            w2 = cpool.tile([P, P * NG], f32r, tag="c_w2")
            nc.scalar.dma_start(out=w2[:], in_=w2_in[:])
            w3 = cpool.tile([P, NSLOT * NG], f32r, tag="c_w3")
            nc.sync.dma_start(out=w3[:], in_=w3_in[:])
            wmaskp = cpool.tile([NSLOT, KT], f32, tag="c_wm")
            nc.sync.dma_start(out=wmaskp[:], in_=wm_in[:])
            permM = cpool.tile([P, P], f32r, tag="c_pm")
            nc.sync.dma_start(out=permM[:], in_=pm_in[:])
            b3c = cpool.tile([NSLOT, 1], f32, tag="c_b3")
            nc.sync.dma_start(out=b3c[:], in_=b3_in[:])
            ones2 = cpool.tile([NSLOT, 2], f32r, tag="c_o12")
            nc.gpsimd.dma_start(out=ones2[:], in_=on_in[:])

            sc1 = pconst[:, 0:NG]
            bi1 = pconst[:, NG:2 * NG]
            b2c = pconst[:, 2 * NG:3 * NG]
            keep = pconst[:, 9:10]

            # ---- phase B: knot values ----
            h1 = [cpool.tile([P, KT], f32r, tag=f"h1_{g}", name=f"h1_{g}")
                  for g in range(NG)]
            h2 = [cpool.tile([P, KT], f32r, tag=f"h2_{g}", name=f"h2_{g}")
                  for g in range(NG)]
            w24 = cpool.tile([64, KT], f32, tag="w24")
            h2ps = psum.tile([P, NG * KT], f32, tag="h2ps")
            nc.scalar.activation(out=h1[0][:], in_=knots[:], func=Act.Tanh,
                                 bias=bi1[:, 0:1], scale=sc1[:, 0:1])
            nc.scalar.activation(out=h1[1][:], in_=knots[:], func=Act.Tanh,
                                 bias=bi1[:, 1:2], scale=sc1[:, 1:2])
            nc.tensor.matmul(out=h2ps[:, 0:KT], lhsT=w2[:, 0:P], rhs=h1[0][:],
                             start=True, stop=True)
            nc.scalar.activation(out=h1[2][:], in_=knots[:], func=Act.Tanh,
                                 bias=bi1[:, 2:3], scale=sc1[:, 2:3])
            nc.tensor.matmul(out=h2ps[:, KT:2 * KT], lhsT=w2[:, P:2 * P],
                             rhs=h1[1][:], start=True, stop=True)
            nc.scalar.activation(out=w24[:], in_=k24[:, 0:KT], func=Act.Tanh,
                                 bias=k24[:, KT + 1:KT + 2],
                                 scale=k24[:, KT:KT + 1])
            nc.tensor.matmul(out=h2ps[:, 2 * KT:3 * KT], lhsT=w2[:, 2 * P:3 * P],
                             rhs=h1[2][:], start=True, stop=True)
            pre = psum.tile([NSLOT, KT], f32, tag="pre")
            for g in range(NG):
                nc.scalar.activation(out=h2[g][:],
                                     in_=h2ps[:, g * KT:(g + 1) * KT],
                                     func=Act.Tanh, bias=b2c[:, g:g + 1],
                                     scale=1.0)
                nc.tensor.matmul(out=pre[:],
                                 lhsT=w3[:, g * NSLOT:(g + 1) * NSLOT],
                                 rhs=h2[g][:], start=(g == 0), stop=(g == 2))

            # window = 0.25*(1+t1)*(1+t2), then *wmaskp (mask * tanh(knot))
            ws1 = wpool.tile([NSLOT, KT], f32, tag="ws1")
            nc.vector.tensor_scalar(out=ws1[:], in0=w24[0:NSLOT, :],
                                    scalar1=1.0, scalar2=0.25,
                                    op0=Op.add, op1=Op.mult)
            ws2 = wpool.tile([NSLOT, KT], f32, tag="ws2")
            nc.vector.tensor_scalar(out=ws2[:], in0=w24[32:32 + NSLOT, :],
                                    scalar1=1.0, scalar2=None, op0=Op.add)
            win = wpool.tile([NSLOT, KT], f32, tag="win")
            nc.vector.tensor_mul(out=win[:], in0=ws1[:], in1=ws2[:])
            nc.vector.tensor_mul(out=win[:], in0=win[:], in1=wmaskp[:])
            term = wpool.tile([NSLOT, KT], f32, tag="term")
            nc.vector.tensor_scalar(out=term[:], in0=pre[:], scalar1=b3c[:],
                                    scalar2=None, op0=Op.add)
            full = wpool.tile([NSLOT, KT], f32r, tag="full")
            nc.vector.tensor_mul(out=full[:], in0=term[:], in1=win[:])

            # ---- phase C: per-partition records a, b ----
            redu = psum.tile([P, 8], f32, tag="redu")
            hiP = redu[:, 0:1]     # [u_hi(cell) | left-limits at spares]
            loP = redu[:, 2:3]     # a-record: [u_lo(cell) | right-limits]
            xP = redu[:, 4:5]      # permuted fixups
            nc.tensor.matmul(out=redu[:, 0:2], lhsT=full[:, P:2 * P],
                             rhs=ones2[:], start=True, stop=True)
            nc.tensor.matmul(out=redu[:, 2:4], lhsT=full[:, 0:P],
                             rhs=ones2[:], start=True, stop=True)
            hiS = wpool.tile([P, 2], f32r, tag="hiS")
            nc.vector.tensor_copy(out=hiS[:], in_=hiP.to_broadcast([P, 2]))
            nc.tensor.matmul(out=redu[:, 4:6], lhsT=permM[:], rhs=hiS[:],
                             start=True, stop=True)
            arec = cpool.tile([P, 1], f32, tag="arec")
            nc.vector.tensor_copy(out=arec[:], in_=loP)
            top = wpool.tile([P, 1], f32, tag="top")
            nc.vector.tensor_scalar(out=top[:], in0=hiP, scalar1=keep,
                                    scalar2=None, op0=Op.mult)
            nc.vector.tensor_add(out=top[:], in0=top[:], in1=xP)
            brec = cpool.tile([P, 1], f32, tag="brec")
            nc.vector.tensor_sub(out=brec[:], in0=top[:], in1=arec[:])

            # ---- phase D: y = a + b*tau, one fused op per chunk ----
            for ch in range(NCH_D):
                sl = slice(ch * CHW, (ch + 1) * CHW)
                ybuf = wpool.tile([P, CHW], f32, tag="ybuf", name="ybuf")
                if ch == 0:
                    nc.scalar.activation(out=ybuf[:], in_=tp[:, sl],
                                         func=Act.Identity,
                                         bias=arec[:], scale=brec[:])
                else:
                    nc.vector.tensor_scalar(out=ybuf[:], in0=tp[:, sl],
                                            scalar1=brec[:], scalar2=arec[:],
                                            op0=Op.mult, op1=Op.add)
                nc.gpsimd.dma_start(out=y_out[:, sl], in_=ybuf[:])

    nc.compile()
    _PROGS[S] = nc
    return nc


# ---------------- host-side input prep ----------------------------------------
def _fold_weights(core, W1, b1, W2, b2, W3, b3):
    means, std, mid, Lb, Rb, bnds = _geometry()
    base = DOM0 + core * DW
    act = [w for w in range(NW) if (Rb[w] > base) and (Lb[w] < base + DW)]
    assert len(act) <= NSLOT, f"core {core}: {len(act)} active windows"
    sc1 = np.zeros((P, NG), np.float32)
    bi1 = np.zeros((P, NG), np.float32)
    w2blk = np.zeros((P, P * NG), np.float32)
    w3f = np.zeros((P, NSLOT * NG), np.float32)
    b2c = np.zeros((P, NG), np.float32)
    b3c = np.zeros((NSLOT, 1), np.float32)
    winsc = np.zeros((64, 1), np.float32)
    winbi = np.zeros((64, 1), np.float32)
    for slot, w in enumerate(act):
        g, s = divmod(slot, 4)
        rows = slice(32 * s, 32 * s + 32)
        w1r = W1[w, 0, :].astype(np.float64)
        sc1[rows, g] = (w1r / std[w]).astype(np.float32)
        bi1[rows, g] = (b1[w] - w1r * means[w] / std[w]).astype(np.float32)
        w2blk[rows, g * P + 32 * s: g * P + 32 * s + 32] = W2[w]
        w3f[rows, g * NSLOT + slot] = W3[w, :, 0]
        b2c[rows, g] = b2[w]
        b3c[slot, 0] = b3[w, 0]
        winsc[slot, 0] = 0.5
        winbi[slot, 0] = np.float32(-0.5 * float(mid[w]))
        winsc[32 + slot, 0] = -0.5
        winbi[32 + slot, 0] = np.float32(0.5 * float(mid[w + 1]))
    return sc1, bi1, w2blk, w3f, b2c, b3c, winsc, winbi


def _core_straddles(core):
    """Per-core straddle boundaries: list of (fp32 boundary, global cell)."""
    means, std, mid, Lb, Rb, bnds = _geometry()
    base = DOM0 + core * DW
    out = []
    for b in bnds:
        if base <= b < base + DW:
            bf = np.float32(b)
            jg = int(np.floor(float(bf) / H))
            out.append((bf, jg))
    assert len(out) <= NB
    return out


_TABLES = None


def _core_tables(core):
    """Input-independent per-core constant tables (cached)."""
    global _TABLES
    if _TABLES is None:
        _TABLES = {}
    if core in _TABLES:
        return _TABLES[core]
    means, std, mid, Lb, Rb, bnds = _geometry()
    base = DOM0 + core * DW
    endx = np.float32(base + DW)
    kidx = np.arange(CL + 1, dtype=np.float64)
    kx = (base + kidx * H).astype(np.float32)     # knots 0..120
    knot_row = np.full(KT, endx, np.float32)
    knot_row[0:CL] = kx[0:CL]                     # c0..c119: knots 0..119
    knot_row[P:P + CL] = kx[1:CL + 1]             # c128..c247: knots 1..120
    strads = _core_straddles(core)
    keep = np.zeros((P, 1), np.float32)
    keep[0:CL, 0] = 1.0
    pm = np.zeros((P, P), np.float32)
    for k, (bf, jg) in enumerate(strads):
        j = jg - core * CL
        assert 0 <= j < CL
        knot_row[CL + k] = bf                              # specP (right limit)
        knot_row[P + CL + k] = np.nextafter(bf, np.float32(-np.inf))  # specM
        keep[j, 0] = 0.0
        pm[CL + k, j] = 1.0       # left-limit (at spare slot of hiP) -> cell j
        pm[j, CL + k] = 1.0       # u_hi[j] -> spare partition
    knotrep = np.broadcast_to(knot_row, (P, KT)).copy()
    k24row = np.broadcast_to(knot_row, (64, KT)).copy()
    # wmaskp = window mask at knots * tanh(knot): the final ansatz folded in
    act = [w for w in range(NW) if (Rb[w] > base) and (Lb[w] < base + DW)]
    th = np.tanh(knot_row.astype(np.float64))
    wmaskp = np.zeros((NSLOT, KT), np.float32)
    for slot, w in enumerate(act):
        lbv = np.nextafter(Lb[w], -np.inf)
        m = (knot_row > lbv) & (knot_row < Rb[w])
        wmaskp[slot] = (m * th).astype(np.float32)
    out = (knotrep, k24row, wmaskp, keep, pm)
    _TABLES[core] = out
    return out


_PTMAPS = None


def _point_maps():
    """Global per-cell straddle arrays for the host tau/row mapping."""
    global _PTMAPS
    if _PTMAPS is not None:
        return _PTMAPS
    ncell = NCORES * CL
    tBa = np.full(ncell, 2.0)
    spare = np.zeros(ncell, np.int64)
    isstr = np.zeros(ncell, bool)
    rowbase = np.zeros(ncell, np.int64)
    for core in range(NCORES):
        for k, (bf, jg) in enumerate(_core_straddles(core)):
            isstr[jg] = True
            tBa[jg] = float(bf) / H - jg
            spare[jg] = core * P + CL + k
        lc = np.arange(CL)
        rowbase[core * CL:(core + 1) * CL] = core * P + lc
    _PTMAPS = (tBa, spare, isstr, rowbase)
    return _PTMAPS


def _prep_in_maps(inputs, S):
    x = np.asarray(inputs["x"], np.float32)
    W1 = np.asarray(inputs["W1"], np.float32)
    b1 = np.asarray(inputs["b1"], np.float32)
    W2 = np.asarray(inputs["W2"], np.float32)
    b2 = np.asarray(inputs["b2"], np.float32)
    W3 = np.asarray(inputs["W3"], np.float32)
    b3 = np.asarray(inputs["b3"], np.float32)

    tBa, spare, isstr, rowbase = _point_maps()
    g64 = x.astype(np.float64) / H
    cg = np.minimum(g64.astype(np.int64), NCORES * CL - 1)
    t = g64 - cg
    tb = tBa[cg]
    sstr = isstr[cg]
    sideR = t >= tb
    row = np.where(sstr & sideR, spare[cg], rowbase[cg])
    tau = np.where(sstr, np.where(sideR, (t - tb) / (1.0 - tb), t / tb), t)
    tau = tau.astype(np.float32)

    order = np.argsort(row, kind="stable")
    rs = row[order]
    cnt = np.bincount(row, minlength=NCORES * P)
    maxcnt = int(cnt.max())
    if maxcnt > S:
        raise OverflowError(maxcnt)
    starts = np.concatenate(([0], np.cumsum(cnt)))
    rank = np.arange(len(x)) - starts[rs]           # rank within own row
    slot = rs * S + rank                            # global padded slot index

    in_maps = []
    for core in range(NCORES):
        tpad = np.zeros(P * S, np.float32)          # pad tau=0 -> y=a (finite)
        msk = (rs >= core * P) & (rs < (core + 1) * P)
        tpad[slot[msk] - core * P * S] = tau[order[msk]]
        sc1, bi1, w2blk, w3f, b2c, b3c, winsc, winbi = _fold_weights(
            core, W1, b1, W2, b2, W3, b3)
        knotrep, k24row, wmaskp, keep, pm = _core_tables(core)
        pconst = np.concatenate([sc1, bi1, b2c, keep], axis=1)
        k24sb = np.concatenate([k24row, winsc, winbi], axis=1)
        in_maps.append({
            "t_pts": tpad.reshape(P, S),
            "k24sb": k24sb,
            "knotrep": knotrep,
            "pconst": pconst,
            "w2blk": w2blk,
            "w3f": w3f,
            "b3c": b3c,
            "wmaskp": wmaskp,
            "permM": pm,
            "ones2r": np.ones((NSLOT, 2), np.float32),
        })
    return in_maps, order, slot


def _unpack(results, order, slot, n_total):
    allys = np.concatenate([r["y_out"].reshape(-1) for r in results])
    out = np.empty(n_total, np.float32)
    out[order] = allys[slot]
    return out


def kernel(**inputs) -> np.ndarray:
    from concourse.bass_utils import run_bass_kernel_spmd

    S = S_DEFAULT
    while True:
        try:
            in_maps, order, slot = _prep_in_maps(inputs, S)
            break
        except OverflowError as e:
            S = ((int(e.args[0]) + 2 * NCH_D - 1) // (2 * NCH_D)) * (2 * NCH_D)
    nc = _build_program(S)
    res = run_bass_kernel_spmd(nc, in_maps, list(range(NCORES)))
    return _unpack(res.results, order, slot, len(np.asarray(inputs["x"])))


# revision 15
# speedup vs baseline: 4.4640x; 1.1261x over previous
"""FBPinn forward kernel for Trainium2 (8 NeuronCores, Bass/Tile).

y(x) = tanh(x) * sum_w [win_w(x)>1e-3] * win_w(x) * MLP_w(x) for 1M points.
Strategy: tabulate the scalar function on a coarse uniform grid (PL error
~2e-3 vs the 2e-2 gate) and interpolate; all discontinuity handling is
resolved on the host.

Layout: 120 grid cells + 8 spare partitions per core (cell = partition).
Straddle cells (window-mask flips inside the cell) keep their left segment;
right-segment points are repacked to a spare partition. The host sends each
point's segment-normalized coordinate tau in [0,1), so every partition's
answer is y = a + b*tau with per-partition scalars a, b:
  B. evaluate the function at 256 knot columns (120 knots | 8 right-limits |
     120 shifted knots | 8 left-limits) via 3 block-diag f32r matmuls + tanh
     on ACT; window sigmoids via tanh (single ACT table); window mask and the
     tanh(x) ansatz folded into one host constant.
  C. records fully on-chip: slot-reduce matmuls give the a-column and the
     hi-column directly ([128,1]); one constant permutation matmul swaps
     (left-limits -> straddle cells, cell hi -> spares); b = hi' - a.
  D. per chunk, one fused op: tensor_scalar(t*b+a) on DVE / Identity ACT on
     Scalar. No compares, no selects.
"""

import numpy as np

# ---------------- problem constants (hardcoded from the module spec) ----------
NW = 30
DOM0, DOM1 = 0.0, 100.0
OVERLAP = 0.25
NEURONS = 32
THRESH = 0.001
N = 1_000_000

NCORES = 8
P = 128                      # SBUF partitions
CL = 120                     # grid cells per core (partitions 120..127 spare)
DW = 12.5                    # per-core domain width
H = DW / CL                  # global cell width
NG = 3                       # window groups of 4 per core
NSLOT = 4 * NG               # window slots per core
NB = 8                       # straddle-boundary slots per core
KT = 256                     # knot columns
S_DEFAULT = 1168             # point slots per partition
NCH_D = 4                    # phase-D chunks


# ---------------- geometry (host, input-independent) --------------------------
def _partition_geom():
    width = (DOM1 - DOM0) / NW
    sub = np.zeros((NW, 2), np.float32)
    for i in range(NW):
        sub[i, 0] = DOM0 if i == 0 else DOM0 + (i - OVERLAP / 2) * width
        sub[i, 1] = DOM1 if i == NW - 1 else DOM0 + (i + 1 + OVERLAP / 2) * width
    means = (sub[:, 0] + sub[:, 1]) / 2
    std = (sub[:, 1] - sub[:, 0]) / 2
    mid = np.zeros(NW + 1, np.float32)
    mid[0] = sub[0, 0]
    mid[-1] = sub[-1, 1]
    for i in range(1, NW):
        mid[i] = (sub[i - 1, 1] + sub[i, 0]) / 2
    return means.astype(np.float32), std.astype(np.float32), mid.astype(np.float32)


def _win64(l, r, x):
    return 1.0 / (1 + np.exp(-(x - l))) / (1 + np.exp(x - r))


def _bisect64(l, r, lo, hi, rising):
    for _ in range(200):
        m = 0.5 * (lo + hi)
        if (_win64(l, r, m) < THRESH) == rising:
            lo = m
        else:
            hi = m
    return 0.5 * (lo + hi)


def _refine_flip_fp32(l32, r32, b64, rising):
    """Exact fp32 x where the reference's jax-fp32 predicate win(x)>1e-3 flips.
    Returns the smallest fp32 x at which the predicate equals its right-side
    state. Falls back to the float64 bisection value if jax is unavailable."""
    try:
        import jax
        import jax.numpy as jnp

        cpu = jax.devices("cpu")[0]
        lo = np.float32(b64 - 5e-5)
        hi = np.float32(b64 + 5e-5)
        xs = np.arange(lo.view(np.int32), hi.view(np.int32) + 1,
                       dtype=np.int32).view(np.float32)
        with jax.default_device(cpu):
            win = np.asarray(
                jax.nn.sigmoid(jnp.asarray(xs) - np.float32(l32))
                * jax.nn.sigmoid(-(jnp.asarray(xs) - np.float32(r32)))
            )
        pred = win > np.float32(THRESH)
        state = pred if rising else ~pred
        if not state.any() or state.all():
            return np.float32(b64)
        k = int(np.argmax(state))
        if not state[k:].all():
            return np.float32(b64)
        return xs[k]
    except Exception:
        return np.float32(b64)


_GEOM = None


def _geometry():
    global _GEOM
    if _GEOM is not None:
        return _GEOM
    means, std, mid = _partition_geom()
    ml = mid[:-1].astype(np.float64)
    mr = mid[1:].astype(np.float64)
    Lb = np.zeros(NW, np.float32)   # window-on lower bound (exact fp32 flip)
    Rb = np.zeros(NW, np.float32)   # window-off upper bound
    for w in range(NW):
        c = 0.5 * (ml[w] + mr[w])
        l64 = _bisect64(ml[w], mr[w], ml[w] - 30, c, rising=True)
        r64 = _bisect64(ml[w], mr[w], c, mr[w] + 30, rising=False)
        Lb[w] = _refine_flip_fp32(mid[w], mid[w + 1], l64, rising=True)
        Rb[w] = _refine_flip_fp32(mid[w], mid[w + 1], r64, rising=False)
    bnds = []
    for w in range(NW):
        if DOM0 < Lb[w] < DOM1:
            bnds.append(float(Lb[w]))
        if DOM0 < Rb[w] < DOM1:
            bnds.append(float(Rb[w]))
    bnds = np.sort(np.array(bnds, np.float64))
    _GEOM = (means, std, mid, Lb, Rb, bnds)
    return _GEOM


# ---------------- bass program (built once per S, SPMD across 8 cores) --------
_PROGS = {}


def _build_program(S):
    if S in _PROGS:
        return _PROGS[S]
    from concourse import bacc, mybir, tile

    f32 = mybir.dt.float32
    f32r = mybir.dt.float32r
    Act = mybir.ActivationFunctionType
    Op = mybir.AluOpType

    CHW = S // NCH_D

    nc = bacc.Bacc(None, target_bir_lowering=False)

    t_in = nc.declare_dram_parameter("t_pts", [P, S], f32, isOutput=False)
    k24_in = nc.declare_dram_parameter("k24sb", [64, KT + 2], f32, isOutput=False)
    kr_in = nc.declare_dram_parameter("knotrep", [P, KT], f32, isOutput=False)
    pc_in = nc.declare_dram_parameter("pconst", [P, 10], f32, isOutput=False)
    w2_in = nc.declare_dram_parameter("w2blk", [P, P * NG], f32r, isOutput=False)
    w3_in = nc.declare_dram_parameter("w3f", [P, NSLOT * NG], f32r, isOutput=False)
    b3_in = nc.declare_dram_parameter("b3c", [NSLOT, 1], f32, isOutput=False)
    wm_in = nc.declare_dram_parameter("wmaskp", [NSLOT, KT], f32, isOutput=False)
    pm_in = nc.declare_dram_parameter("permM", [P, P], f32r, isOutput=False)
    on_in = nc.declare_dram_parameter("ones2r", [NSLOT, 2], f32r, isOutput=False)
    y_out = nc.declare_dram_parameter("y_out", [P, S], f32, isOutput=True)

    with tile.TileContext(nc) as tc:
        with (
            tc.tile_pool(name="const", bufs=1) as cpool,
            tc.tile_pool(name="work", bufs=2) as wpool,
            tc.tile_pool(name="psum", bufs=1, space="PSUM") as psum,
        ):
            # ---- constant loads (small critical tables first per queue) ----
            knots = cpool.tile([P, KT], f32, tag="c_kr")
            nc.gpsimd.dma_start(out=knots[:], in_=kr_in[:])
            pconst = cpool.tile([P, 10], f32, tag="c_pc")
            nc.scalar.dma_start(out=pconst[:], in_=pc_in[:])
            k24 = cpool.tile([64, KT + 2], f32, tag="c_k24")
            nc.sync.dma_start(out=k24[:], in_=k24_in[:])
            w2 = cpool.tile([P, P * NG], f32r, tag="c_w2")
            nc.scalar.dma_start(out=w2[:], in_=w2_in[:])
            tp = cpool.tile([P, S], f32, tag="c_t")
            nc.gpsimd.dma_start(out=tp[:], in_=t_in[:])
            w3 = cpool.tile([P, NSLOT * NG], f32r, tag="c_w3")
            nc.sync.dma_start(out=w3[:], in_=w3_in[:])
            wmaskp = cpool.tile([NSLOT, KT], f32, tag="c_wm")
            nc.sync.dma_start(out=wmaskp[:], in_=wm_in[:])
            permM = cpool.tile([P, P], f32r, tag="c_pm")
            nc.gpsimd.dma_start(out=permM[:], in_=pm_in[:])
            ones2 = cpool.tile([NSLOT, 2], f32r, tag="c_o12")
            nc.gpsimd.dma_start(out=ones2[:], in_=on_in[:])
            b3c = cpool.tile([NSLOT, 1], f32, tag="c_b3")
            nc.sync.dma_start(out=b3c[:], in_=b3_in[:])

            sc1 = pconst[:, 0:NG]
            bi1 = pconst[:, NG:2 * NG]
            b2c = pconst[:, 2 * NG:3 * NG]
            keep = pconst[:, 9:10]

            # ---- phase B: knot values ----
            h1 = [cpool.tile([P, KT], f32r, tag=f"h1_{g}", name=f"h1_{g}")
                  for g in range(NG)]
            h2 = [cpool.tile([P, KT], f32r, tag=f"h2_{g}", name=f"h2_{g}")
                  for g in range(NG)]
            w24 = cpool.tile([64, KT], f32, tag="w24")
            h2ps = psum.tile([P, NG * KT], f32, tag="h2ps")
            nc.scalar.activation(out=h1[0][:], in_=knots[:], func=Act.Tanh,
                                 bias=bi1[:, 0:1], scale=sc1[:, 0:1])
            nc.scalar.activation(out=h1[1][:], in_=knots[:], func=Act.Tanh,
                                 bias=bi1[:, 1:2], scale=sc1[:, 1:2])
            nc.tensor.matmul(out=h2ps[:, 0:KT], lhsT=w2[:, 0:P], rhs=h1[0][:],
                             start=True, stop=True)
            nc.scalar.activation(out=h1[2][:], in_=knots[:], func=Act.Tanh,
                                 bias=bi1[:, 2:3], scale=sc1[:, 2:3])
            nc.tensor.matmul(out=h2ps[:, KT:2 * KT], lhsT=w2[:, P:2 * P],
                             rhs=h1[1][:], start=True, stop=True)
            nc.scalar.activation(out=w24[:], in_=k24[:, 0:KT], func=Act.Tanh,
                                 bias=k24[:, KT + 1:KT + 2],
                                 scale=k24[:, KT:KT + 1])
            nc.tensor.matmul(out=h2ps[:, 2 * KT:3 * KT], lhsT=w2[:, 2 * P:3 * P],
                             rhs=h1[2][:], start=True, stop=True)
            pre = psum.tile([NSLOT, KT], f32, tag="pre")
            for g in range(NG):
                nc.scalar.activation(out=h2[g][:],
                                     in_=h2ps[:, g * KT:(g + 1) * KT],
                                     func=Act.Tanh, bias=b2c[:, g:g + 1],
                                     scale=1.0)
                nc.tensor.matmul(out=pre[:],
                                 lhsT=w3[:, g * NSLOT:(g + 1) * NSLOT],
                                 rhs=h2[g][:], start=(g == 0), stop=(g == 2))

            # window = 0.25*(1+t1)*(1+t2), then *wmaskp (mask * tanh(knot))
            ws1 = wpool.tile([NSLOT, KT], f32, tag="ws1")
            nc.vector.tensor_scalar(out=ws1[:], in0=w24[0:NSLOT, :],
                                    scalar1=1.0, scalar2=0.25,
                                    op0=Op.add, op1=Op.mult)
            ws2 = wpool.tile([NSLOT, KT], f32, tag="ws2")
            nc.vector.tensor_scalar(out=ws2[:], in0=w24[32:32 + NSLOT, :],
                                    scalar1=1.0, scalar2=None, op0=Op.add)
            win = wpool.tile([NSLOT, KT], f32, tag="win")
            nc.vector.tensor_mul(out=win[:], in0=ws1[:], in1=ws2[:])
            nc.vector.tensor_mul(out=win[:], in0=win[:], in1=wmaskp[:])
            term = wpool.tile([NSLOT, KT], f32, tag="term")
            nc.vector.tensor_scalar(out=term[:], in0=pre[:], scalar1=b3c[:],
                                    scalar2=None, op0=Op.add)
            full = wpool.tile([NSLOT, KT], f32r, tag="full")
            nc.vector.tensor_mul(out=full[:], in0=term[:], in1=win[:])

            # ---- phase C: per-partition records a, b ----
            redu = psum.tile([P, 8], f32, tag="redu")
            hiP = redu[:, 0:1]     # [u_hi(cell) | left-limits at spares]
            loP = redu[:, 2:3]     # a-record: [u_lo(cell) | right-limits]
            xP = redu[:, 4:5]      # permuted fixups
            nc.tensor.matmul(out=redu[:, 0:2], lhsT=full[:, P:2 * P],
                             rhs=ones2[:], start=True, stop=True)
            nc.tensor.matmul(out=redu[:, 2:4], lhsT=full[:, 0:P],
                             rhs=ones2[:], start=True, stop=True)
            hiS = wpool.tile([P, 2], f32r, tag="hiS")
            nc.vector.tensor_copy(out=hiS[:], in_=hiP.to_broadcast([P, 2]))
            nc.tensor.matmul(out=redu[:, 4:6], lhsT=permM[:], rhs=hiS[:],
                             start=True, stop=True)
            arec = cpool.tile([P, 1], f32, tag="arec")
            nc.vector.tensor_copy(out=arec[:], in_=loP)
            top = wpool.tile([P, 1], f32, tag="top")
            nc.vector.tensor_scalar(out=top[:], in0=hiP, scalar1=keep,
                                    scalar2=None, op0=Op.mult)
            nc.vector.tensor_add(out=top[:], in0=top[:], in1=xP)
            brec = cpool.tile([P, 1], f32, tag="brec")
            nc.vector.tensor_sub(out=brec[:], in0=top[:], in1=arec[:])

            # ---- phase D: y = a + b*tau, one fused op per chunk ----
            for ch in range(NCH_D):
                sl = slice(ch * CHW, (ch + 1) * CHW)
                ybuf = wpool.tile([P, CHW], f32, tag=f"ybuf{ch}",
                                  name=f"ybuf{ch}")
                if ch == 0:
                    nc.scalar.activation(out=ybuf[:], in_=tp[:, sl],
                                         func=Act.Identity,
                                         bias=arec[:], scale=brec[:])
                else:
                    nc.vector.tensor_scalar(out=ybuf[:], in0=tp[:, sl],
                                            scalar1=brec[:], scalar2=arec[:],
                                            op0=Op.mult, op1=Op.add)
                eng = nc.gpsimd if ch % 2 == 0 else nc.sync
                eng.dma_start(out=y_out[:, sl], in_=ybuf[:])

    nc.compile()
    _PROGS[S] = nc
    return nc


# ---------------- host-side input prep ----------------------------------------
def _fold_weights(core, W1, b1, W2, b2, W3, b3):
    means, std, mid, Lb, Rb, bnds = _geometry()
    base = DOM0 + core * DW
    act = [w for w in range(NW) if (Rb[w] > base) and (Lb[w] < base + DW)]
    assert len(act) <= NSLOT, f"core {core}: {len(act)} active windows"
    sc1 = np.zeros((P, NG), np.float32)
    bi1 = np.zeros((P, NG), np.float32)
    w2blk = np.zeros((P, P * NG), np.float32)
    w3f = np.zeros((P, NSLOT * NG), np.float32)
    b2c = np.zeros((P, NG), np.float32)
    b3c = np.zeros((NSLOT, 1), np.float32)
    winsc = np.zeros((64, 1), np.float32)
    winbi = np.zeros((64, 1), np.float32)
    for slot, w in enumerate(act):
        g, s = divmod(slot, 4)
        rows = slice(32 * s, 32 * s + 32)
        w1r = W1[w, 0, :].astype(np.float64)
        sc1[rows, g] = (w1r / std[w]).astype(np.float32)
        bi1[rows, g] = (b1[w] - w1r * means[w] / std[w]).astype(np.float32)
        w2blk[rows, g * P + 32 * s: g * P + 32 * s + 32] = W2[w]
        w3f[rows, g * NSLOT + slot] = W3[w, :, 0]
        b2c[rows, g] = b2[w]
        b3c[slot, 0] = b3[w, 0]
        winsc[slot, 0] = 0.5
        winbi[slot, 0] = np.float32(-0.5 * float(mid[w]))
        winsc[32 + slot, 0] = -0.5
        winbi[32 + slot, 0] = np.float32(0.5 * float(mid[w + 1]))
    return sc1, bi1, w2blk, w3f, b2c, b3c, winsc, winbi


def _core_straddles(core):
    """Per-core straddle boundaries: list of (fp32 boundary, global cell)."""
    means, std, mid, Lb, Rb, bnds = _geometry()
    base = DOM0 + core * DW
    out = []
    for b in bnds:
        if base <= b < base + DW:
            bf = np.float32(b)
            jg = int(np.floor(float(bf) / H))
            out.append((bf, jg))
    assert len(out) <= NB
    return out


_TABLES = None


def _core_tables(core):
    """Input-independent per-core constant tables (cached)."""
    global _TABLES
    if _TABLES is None:
        _TABLES = {}
    if core in _TABLES:
        return _TABLES[core]
    means, std, mid, Lb, Rb, bnds = _geometry()
    base = DOM0 + core * DW
    endx = np.float32(base + DW)
    kidx = np.arange(CL + 1, dtype=np.float64)
    kx = (base + kidx * H).astype(np.float32)     # knots 0..120
    knot_row = np.full(KT, endx, np.float32)
    knot_row[0:CL] = kx[0:CL]                     # c0..c119: knots 0..119
    knot_row[P:P + CL] = kx[1:CL + 1]             # c128..c247: knots 1..120
    strads = _core_straddles(core)
    keep = np.zeros((P, 1), np.float32)
    keep[0:CL, 0] = 1.0
    pm = np.zeros((P, P), np.float32)
    for k, (bf, jg) in enumerate(strads):
        j = jg - core * CL
        assert 0 <= j < CL
        knot_row[CL + k] = bf                              # specP (right limit)
        knot_row[P + CL + k] = np.nextafter(bf, np.float32(-np.inf))  # specM
        keep[j, 0] = 0.0
        pm[CL + k, j] = 1.0       # left-limit (at spare slot of hiP) -> cell j
        pm[j, CL + k] = 1.0       # u_hi[j] -> spare partition
    knotrep = np.broadcast_to(knot_row, (P, KT)).copy()
    k24row = np.broadcast_to(knot_row, (64, KT)).copy()
    # wmaskp = window mask at knots * tanh(knot): the final ansatz folded in
    act = [w for w in range(NW) if (Rb[w] > base) and (Lb[w] < base + DW)]
    th = np.tanh(knot_row.astype(np.float64))
    wmaskp = np.zeros((NSLOT, KT), np.float32)
    for slot, w in enumerate(act):
        lbv = np.nextafter(Lb[w], -np.inf)
        m = (knot_row > lbv) & (knot_row < Rb[w])
        wmaskp[slot] = (m * th).astype(np.float32)
    out = (knotrep, k24row, wmaskp, keep, pm)
    _TABLES[core] = out
    return out


_PTMAPS = None


def _point_maps():
    """Global per-cell straddle arrays for the host tau/row mapping."""
    global _PTMAPS
    if _PTMAPS is not None:
        return _PTMAPS
    ncell = NCORES * CL
    tBa = np.full(ncell, 2.0)
    spare = np.zeros(ncell, np.int64)
    isstr = np.zeros(ncell, bool)
    rowbase = np.zeros(ncell, np.int64)
    for core in range(NCORES):
        for k, (bf, jg) in enumerate(_core_straddles(core)):
            isstr[jg] = True
            tBa[jg] = float(bf) / H - jg
            spare[jg] = core * P + CL + k
        lc = np.arange(CL)
        rowbase[core * CL:(core + 1) * CL] = core * P + lc
    _PTMAPS = (tBa, spare, isstr, rowbase)
    return _PTMAPS


def _prep_in_maps(inputs, S):
    x = np.asarray(inputs["x"], np.float32)
    W1 = np.asarray(inputs["W1"], np.float32)
    b1 = np.asarray(inputs["b1"], np.float32)
    W2 = np.asarray(inputs["W2"], np.float32)
    b2 = np.asarray(inputs["b2"], np.float32)
    W3 = np.asarray(inputs["W3"], np.float32)
    b3 = np.asarray(inputs["b3"], np.float32)

    tBa, spare, isstr, rowbase = _point_maps()
    g64 = x.astype(np.float64) / H
    cg = np.minimum(g64.astype(np.int64), NCORES * CL - 1)
    t = g64 - cg
    tb = tBa[cg]
    sstr = isstr[cg]
    sideR = t >= tb
    row = np.where(sstr & sideR, spare[cg], rowbase[cg])
    tau = np.where(sstr, np.where(sideR, (t - tb) / (1.0 - tb), t / tb), t)
    tau = tau.astype(np.float32)

    order = np.argsort(row, kind="stable")
    rs = row[order]
    cnt = np.bincount(row, minlength=NCORES * P)
    maxcnt = int(cnt.max())
    if maxcnt > S:
        raise OverflowError(maxcnt)
    starts = np.concatenate(([0], np.cumsum(cnt)))
    rank = np.arange(len(x)) - starts[rs]           # rank within own row
    slot = rs * S + rank                            # global padded slot index

    in_maps = []
    for core in range(NCORES):
        tpad = np.zeros(P * S, np.float32)          # pad tau=0 -> y=a (finite)
        msk = (rs >= core * P) & (rs < (core + 1) * P)
        tpad[slot[msk] - core * P * S] = tau[order[msk]]
        sc1, bi1, w2blk, w3f, b2c, b3c, winsc, winbi = _fold_weights(
            core, W1, b1, W2, b2, W3, b3)
        knotrep, k24row, wmaskp, keep, pm = _core_tables(core)
        pconst = np.concatenate([sc1, bi1, b2c, keep], axis=1)
        k24sb = np.concatenate([k24row, winsc, winbi], axis=1)
        in_maps.append({
            "t_pts": tpad.reshape(P, S),
            "k24sb": k24sb,
            "knotrep": knotrep,
            "pconst": pconst,
            "w2blk": w2blk,
            "w3f": w3f,
            "b3c": b3c,
            "wmaskp": wmaskp,
            "permM": pm,
            "ones2r": np.ones((NSLOT, 2), np.float32),
        })
    return in_maps, order, slot


def _unpack(results, order, slot, n_total):
    allys = np.concatenate([r["y_out"].reshape(-1) for r in results])
    out = np.empty(n_total, np.float32)
    out[order] = allys[slot]
    return out


def kernel(**inputs) -> np.ndarray:
    from concourse.bass_utils import run_bass_kernel_spmd

    S = S_DEFAULT
    while True:
        try:
            in_maps, order, slot = _prep_in_maps(inputs, S)
            break
        except OverflowError as e:
            S = ((int(e.args[0]) + 2 * NCH_D - 1) // (2 * NCH_D)) * (2 * NCH_D)
    nc = _build_program(S)
    res = run_bass_kernel_spmd(nc, in_maps, list(range(NCORES)))
    return _unpack(res.results, order, slot, len(np.asarray(inputs["x"])))
